# revision 1
# baseline (speedup 1.0000x reference)
"""DGCNN classifier kernel for 8 Trainium2 NeuronCores.

Strategy (per sharding hint): data-parallel over batch B=8, one sample per
NeuronCore, all weights replicated. Each core runs the full per-sample
DGCNN chain:
  4x EdgeConv (kNN top-20 on the pairwise-distance matrix + 1x1 conv +
  BN + LeakyReLU + max over neighbors), 1x1 conv to 1024, global max+mean
  pooling, and 3 FC layers.

Per-core math uses an algebraic reduction of EdgeConv: with W = [Wc | Wd]
split over the (center, nbr-center) channel halves,
    y[o,n,j] = ((Wc-Wd) @ x)[o,n] + (Wd @ x)[o, idx[n,j]]
so the [N, k, 2C] edge-feature tensor and its O x 2C x N x k einsum are
never materialized; only two [O, C] @ [C, N] matmuls plus a gather+max
remain (~20x fewer conv FLOPs than the reference formulation). BN+LeakyReLU
fold to a per-channel scale/bias; max-over-neighbors commutes through the
monotone BN+LeakyReLU when the folded scale is positive (verified against
the actual weights at call time; channels with negative scale fall back to
an exact min-based path).

Inputs arrive as full (unsharded) numpy arrays; output is the full [8, 40]
logits array. Sharding/gather happens inside via jax.pmap over the 8 cores.
"""

import numpy as np
import jax
import jax.numpy as jnp

EPS = 1e-5
K = 20
N_CORES = 8

_WEIGHT_KEYS = [
    "W1", "bn1_g", "bn1_b", "bn1_m", "bn1_v",
    "W2", "bn2_g", "bn2_b", "bn2_m", "bn2_v",
    "W3", "bn3_g", "bn3_b", "bn3_m", "bn3_v",
    "W4", "bn4_g", "bn4_b", "bn4_m", "bn4_v",
    "W5", "bn5_g", "bn5_b", "bn5_m", "bn5_v",
    "L1", "bn6_g", "bn6_b", "bn6_m", "bn6_v",
    "L2", "bn7_g", "bn7_b", "bn7_m", "bn7_v",
    "L3", "L3_b",
]


def _bn_fold(g, b, m, v):
    s = g * jax.lax.rsqrt(v + EPS)
    return s, b - m * s


def _lrelu(x):
    return jnp.where(x > 0, x, 0.2 * x)


def _edgeconv(x, W, g, b, m, v, all_pos):
    """x: [C, N] -> [O, N]. all_pos: static flag, True when every folded BN
    scale is positive so max commutes through BN+LeakyReLU directly."""
    C, N = x.shape
    xt = x.T                                        # [N, C]
    xx = jnp.sum(x * x, axis=0)                     # [N]
    # Same dist expression/op-order as the reference for identical top-k.
    dist = xx[:, None] + xx[None, :] - 2.0 * (xt @ xt.T)
    _, idx = jax.lax.top_k(-dist, K)                # [N, K]
    Wc, Wd = W[:, :C], W[:, C:]
    a = (Wc - Wd) @ x                               # [O, N]
    bmat = Wd @ x                                   # [O, N]
    nbr = bmat.T[idx]                               # [N, K, O]
    s, t = _bn_fold(g, b, m, v)
    if all_pos:
        B = jnp.max(nbr, axis=1).T                  # [O, N]
    else:
        B = jnp.where((s >= 0)[:, None],
                      jnp.max(nbr, axis=1).T, jnp.min(nbr, axis=1).T)
    return _lrelu((a + B) * s[:, None] + t[:, None])


def _forward_one(x, w, all_pos):
    """x: [3, N] one sample; w: dict of replicated weights -> [40] logits."""
    x1 = _edgeconv(x, w["W1"], w["bn1_g"], w["bn1_b"], w["bn1_m"], w["bn1_v"], all_pos)
    x2 = _edgeconv(x1, w["W2"], w["bn2_g"], w["bn2_b"], w["bn2_m"], w["bn2_v"], all_pos)
    x3 = _edgeconv(x2, w["W3"], w["bn3_g"], w["bn3_b"], w["bn3_m"], w["bn3_v"], all_pos)
    x4 = _edgeconv(x3, w["W4"], w["bn4_g"], w["bn4_b"], w["bn4_m"], w["bn4_v"], all_pos)
    xc = jnp.concatenate([x1, x2, x3, x4], axis=0)          # [512, N]
    s5, t5 = _bn_fold(w["bn5_g"], w["bn5_b"], w["bn5_m"], w["bn5_v"])
    emb = _lrelu((w["W5"] @ xc) * s5[:, None] + t5[:, None])  # [1024, N]
    feat = jnp.concatenate([jnp.max(emb, axis=1), jnp.mean(emb, axis=1)])
    s6, t6 = _bn_fold(w["bn6_g"], w["bn6_b"], w["bn6_m"], w["bn6_v"])
    h = _lrelu((w["L1"] @ feat) * s6 + t6)
    s7, t7 = _bn_fold(w["bn7_g"], w["bn7_b"], w["bn7_m"], w["bn7_v"])
    h = _lrelu((w["L2"] @ h) * s7 + t7)
    return w["L3"] @ h + w["L3_b"]


# One compiled pmap per all_pos variant (static python flag).
_PMAPS = {}


def _get_pmap(all_pos):
    if all_pos not in _PMAPS:
        _PMAPS[all_pos] = jax.pmap(
            lambda x, w: _forward_one(x, w, all_pos),
            in_axes=(0, None),
            devices=jax.devices()[:N_CORES],
        )
    return _PMAPS[all_pos]


# Device-resident weight cache: avoids re-uploading ~8 MB of weights over
# the tunnel on every call. Keyed by a cheap fingerprint of the host arrays.
_WCACHE = {}


def _fingerprint(arrs):
    h = 0
    for a in arrs:
        h ^= hash((a.shape, a.dtype.str, a.tobytes()[:64], a.tobytes()[-64:]))
    return h


def kernel(**inputs):
    x = np.ascontiguousarray(np.asarray(inputs["x"], dtype=np.float32))
    assert x.shape[0] == N_CORES, f"expected batch {N_CORES}, got {x.shape}"
    host_w = [np.ascontiguousarray(np.asarray(inputs[k], dtype=np.float32))
              for k in _WEIGHT_KEYS]
    fp = _fingerprint(host_w)
    if fp not in _WCACHE:
        w = {k: jnp.asarray(a) for k, a in zip(_WEIGHT_KEYS, host_w)}
        # max-over-neighbors commutes through BN+LeakyReLU iff scale > 0,
        # i.e. iff g > 0 (rsqrt(v+eps) > 0). Checked on the real weights.
        all_pos = all(float(np.min(inputs[f"bn{i}_g"])) > 0 for i in (1, 2, 3, 4))
        _WCACHE[fp] = (w, all_pos)
    w, all_pos = _WCACHE[fp]
    out = _get_pmap(all_pos)(jnp.asarray(x), w)   # [8, 40], one sample per core
    return np.asarray(out).astype(np.float32)



# revision 2
# speedup vs baseline: 1.8404x; 1.8404x over previous
"""DGCNN classifier for 8 Trainium2 NeuronCores (axon-tunneled).

Strategy: data-parallel over batch B=8 — one sample per core (per the
sharding hint), with ALL weights folded into the compiled executable as
constants. Wall-clock per call through the axon tunnel is dominated by a
~28 ms dispatch/RPC floor; passing the weights as device-resident arrays
adds ~40 ms of per-call arg-resolution RPCs, and passing them as inline
host args adds payload. Baking them as XLA constants removes both, and
lets the compiler pre-fold (Wc-Wd), BN scale/bias, etc.

Per-sample math uses the algebraic EdgeConv reduction: with W = [Wc | Wd]
split over the (center, nbr-center) halves,
    y[o,n,j] = ((Wc-Wd) @ x)[o,n] + (Wd @ x)[o, idx[n,j]]
so only two [O,C]@[C,N] matmuls plus a gather+max remain. BN+LeakyReLU
fold into a per-channel scale/bias applied after the max (the folded
scales are all positive for these weights, so max commutes; verified at
fold time with an exact fallback).
"""

import numpy as np
import jax
import jax.numpy as jnp

EPS = 1e-5
K = 20
N_CORES = 8

_WEIGHT_KEYS = [
    "W1", "bn1_g", "bn1_b", "bn1_m", "bn1_v",
    "W2", "bn2_g", "bn2_b", "bn2_m", "bn2_v",
    "W3", "bn3_g", "bn3_b", "bn3_m", "bn3_v",
    "W4", "bn4_g", "bn4_b", "bn4_m", "bn4_v",
    "W5", "bn5_g", "bn5_b", "bn5_m", "bn5_v",
    "L1", "bn6_g", "bn6_b", "bn6_m", "bn6_v",
    "L2", "bn7_g", "bn7_b", "bn7_m", "bn7_v",
    "L3", "L3_b",
]


def _lrelu(y):
    return jnp.where(y > 0, y, 0.2 * y)


def _fold_bn(inputs, i):
    g, b, m, v = (np.asarray(inputs[f"bn{i}_{c}"], np.float64) for c in "gbmv")
    s = g / np.sqrt(v + EPS)
    return s.astype(np.float32), (b - m * s).astype(np.float32)


def _build_forward(inputs):
    """Returns forward_one(x) with all weights embedded as constants."""
    C_ = {}
    all_pos = True
    for i, wk in [(1, "W1"), (2, "W2"), (3, "W3"), (4, "W4")]:
        Wm = np.asarray(inputs[wk], np.float32)
        C = Wm.shape[1] // 2
        s, t = _fold_bn(inputs, i)
        all_pos &= bool(np.min(s) > 0)
        C_[f"A{i}"] = jnp.asarray(Wm[:, :C] - Wm[:, C:])
        C_[f"D{i}"] = jnp.asarray(Wm[:, C:])
        C_[f"s{i}"] = jnp.asarray(s)
        C_[f"t{i}"] = jnp.asarray(t)
    for i, wk in [(5, "W5"), (6, "L1"), (7, "L2")]:
        s, t = _fold_bn(inputs, i)
        C_[f"M{i}"] = jnp.asarray(np.asarray(inputs[wk], np.float32) * s[:, None])
        C_[f"t{i}"] = jnp.asarray(t)
    C_["L3"] = jnp.asarray(np.asarray(inputs["L3"], np.float32))
    C_["L3_b"] = jnp.asarray(np.asarray(inputs["L3_b"], np.float32))

    def edgeconv(x, i):
        xt = x.T
        xx = jnp.sum(x * x, axis=0)
        # Same op order as the reference so top-k picks the same neighbors.
        d = xx[:, None] + xx[None, :] - 2.0 * (xt @ xt.T)
        _, idx = jax.lax.top_k(-d, K)
        a = C_[f"A{i}"] @ x
        bm = C_[f"D{i}"] @ x
        nbr = bm.T[idx]                      # [N, K, O]
        s = C_[f"s{i}"]
        if all_pos:
            B = jnp.max(nbr, axis=1).T
        else:
            B = jnp.where((s >= 0)[:, None],
                          jnp.max(nbr, axis=1).T, jnp.min(nbr, axis=1).T)
        return _lrelu((a + B) * s[:, None] + C_[f"t{i}"][:, None])

    def forward_one(x):
        x1 = edgeconv(x, 1)
        x2 = edgeconv(x1, 2)
        x3 = edgeconv(x2, 3)
        x4 = edgeconv(x3, 4)
        xc = jnp.concatenate([x1, x2, x3, x4], axis=0)
        emb = _lrelu(C_["M5"] @ xc + C_["t5"][:, None])
        feat = jnp.concatenate([jnp.max(emb, axis=1), jnp.mean(emb, axis=1)])
        h = _lrelu(C_["M6"] @ feat + C_["t6"])
        h = _lrelu(C_["M7"] @ h + C_["t7"])
        return C_["L3"] @ h + C_["L3_b"]

    return forward_one


# Compiled pmap cache keyed by a fingerprint of the weight arrays.
_CACHE = {}


def _fingerprint(arrs):
    h = 0
    for a in arrs:
        h ^= hash((a.shape, a.dtype.str, a.tobytes()[:64], a.tobytes()[-64:]))
    return h


def kernel(**inputs):
    x = np.ascontiguousarray(np.asarray(inputs["x"], dtype=np.float32))
    assert x.shape[0] == N_CORES, f"expected batch {N_CORES}, got {x.shape}"
    host_w = [np.ascontiguousarray(np.asarray(inputs[k], dtype=np.float32))
              for k in _WEIGHT_KEYS]
    fp = _fingerprint(host_w)
    if fp not in _CACHE:
        fwd = _build_forward(inputs)
        _CACHE[fp] = jax.pmap(fwd, devices=jax.devices()[:N_CORES])
    out = _CACHE[fp](x)          # [8, 40]
    return np.asarray(out).astype(np.float32)


# revision 3
# speedup vs baseline: 2.4394x; 1.3255x over previous
"""DGCNN classifier on 8 Trainium2 NeuronCores (axon-tunneled) — Bass kernel.

Data-parallel over batch B=8: one sample per core (per the sharding hint),
weights folded into NEFF constants. Wall-clock through the axon tunnel is
dominated by a ~30 ms RPC floor, so the kernel minimizes on-device time and
per-call payload (only x is shipped, 24 KB/core; logits [40] come back).

Per-core Bass/Tile kernel (one sample [3, 2048], c-major layout, 16 blocks
of 128 points):
  per EdgeConv layer (C -> O):
    phase 1: bmT = x^T (Wd^T s) -> DRAM gather table; aT = x^T (A^T s) + t
    phase 2a (per block): sel[n, m] = 2*x_n.x_m - |x_m|^2 in PSUM (PE),
      then top-20 per row with 3 rounds of DVE max8/max_index/match_replace
      (row-constant -|x_n|^2 dropped: it does not change per-row order)
    fence: gathers wait for all top-k DVE work (SWDGE descriptor generation
      contends with DVE SBUF ports; interleaving costs ~20 us per gather)
    phase 2b (per block): 20 single-index indirect-DMA gathers of bmT rows,
      DVE max-accumulate, + aT, LeakyReLU as max(z, 0.2 z)
    phase 3: PE-transpose y to next layer's c-major input; -|x|^2 row
  then emb = lrelu(W5' xc + t5) (8 o-blocks), feat = [max_n; sum_n],
  FC chain with BN folded, logits [40, 1].

The top-k set matches the reference's top_k(-dist) per row exactly (up to
fp32 rounding): sel = -dist + |x_n|^2, a row-constant shift.

First call: builds + compiles the NEFF (seconds), runs a numpy reference on
sample 0 and falls back to a pure-jax pmap path (also with folded constant
weights) if anything disagrees or fails.
"""

import numpy as np

EPS = 1e-5
K = 20
N = 2048
P = 128
NB = N // P
CH = 512
N_CORES = 8

_WEIGHT_KEYS = [
    "W1", "bn1_g", "bn1_b", "bn1_m", "bn1_v",
    "W2", "bn2_g", "bn2_b", "bn2_m", "bn2_v",
    "W3", "bn3_g", "bn3_b", "bn3_m", "bn3_v",
    "W4", "bn4_g", "bn4_b", "bn4_m", "bn4_v",
    "W5", "bn5_g", "bn5_b", "bn5_m", "bn5_v",
    "L1", "bn6_g", "bn6_b", "bn6_m", "bn6_v",
    "L2", "bn7_g", "bn7_b", "bn7_m", "bn7_v",
    "L3", "L3_b",
]


def _fold_bn(w, i):
    g, b, m, v = (np.asarray(w[f"bn{i}_{c}"], np.float64) for c in "gbmv")
    s = g / np.sqrt(v + EPS)
    return s, b - m * s


# --------------------------------------------------------------------------
# Bass kernel builder
# --------------------------------------------------------------------------

def _build_tc(tc, w, x_d, out_d):
    import concourse.bass as bass
    import concourse.mybir as mybir
    from concourse.masks import make_identity
    from contextlib import ExitStack

    F32 = mybir.dt.float32
    U32 = mybir.dt.uint32
    nc = tc.nc

    LAY = []
    for i, wk in [(1, "W1"), (2, "W2"), (3, "W3"), (4, "W4")]:
        Wm = np.asarray(w[wk], np.float64)
        O, C2 = Wm.shape
        C = C2 // 2
        Wc, Wd = Wm[:, :C], Wm[:, C:]
        s, t = _fold_bn(w, i)
        assert np.min(s) > 0, "need positive folded BN scale (max-commute)"
        WaT = ((Wc - Wd) * s[:, None]).T.astype(np.float32).copy()
        WdT = (Wd * s[:, None]).T.astype(np.float32).copy()
        trow = t.astype(np.float32).reshape(1, O).copy()
        LAY.append((C, O, WaT, trow, WdT))

    s5, t5 = _fold_bn(w, 5)
    W5T = (np.asarray(w["W5"], np.float64) * s5[:, None]).T.astype(np.float32).copy()
    t5cols = t5.astype(np.float32).reshape(8, P).T.copy()
    t5cols08 = (0.8 * t5cols).astype(np.float32).copy()
    s6, t6 = _fold_bn(w, 6)
    L1f = np.asarray(w["L1"], np.float64) * s6[:, None]
    L1f[:, 1024:] = L1f[:, 1024:] / float(N)
    L1T = L1f.T.astype(np.float32).copy()
    t6row = t6.astype(np.float32).reshape(1, 512).copy()
    s7, t7 = _fold_bn(w, 7)
    L2T = (np.asarray(w["L2"], np.float64) * s7[:, None]).T.astype(np.float32).copy()
    t7row = t7.astype(np.float32).reshape(1, 256).copy()
    L3T = np.asarray(w["L3"], np.float32).T.copy()
    L3brow = np.asarray(w["L3_b"], np.float32).reshape(1, 40).copy()

    c_WaT = [nc.inline_tensor(l[2], name=f"WaT{i}") for i, l in enumerate(LAY)]
    c_trow = [nc.inline_tensor(l[3], name=f"trow{i}") for i, l in enumerate(LAY)]
    c_WdT = [nc.inline_tensor(l[4], name=f"WdT{i}") for i, l in enumerate(LAY)]
    c_W5T = nc.inline_tensor(W5T, name="W5T")
    c_t5 = nc.inline_tensor(np.concatenate([t5cols, t5cols08], axis=1), name="t5c")
    c_L1T = nc.inline_tensor(L1T, name="L1T")
    c_t6 = nc.inline_tensor(t6row, name="t6row")
    c_L2T = nc.inline_tensor(L2T, name="L2T")
    c_t7 = nc.inline_tensor(t7row, name="t7row")
    c_L3T = nc.inline_tensor(L3T, name="L3T")
    c_L3b = nc.inline_tensor(L3brow, name="L3brow")
    c_ones = nc.inline_tensor(np.ones((1, N), np.float32), name="onesrow")

    AF = mybir.ActivationFunctionType
    ALU = mybir.AluOpType

    with ExitStack() as ctx:
        glob = ctx.enter_context(tc.tile_pool(name="glob", bufs=1))
        ident = glob.tile([P, P], F32)
        make_identity(nc, ident)
        ones_t = glob.tile([P, N], F32)
        nc.sync.dma_start(out=ones_t[0:1, :], in_=c_ones[:, :])
        ones11 = ones_t[0:1, 0:1]

        feat_pool = ctx.enter_context(tc.tile_pool(name="feat", bufs=1))
        F12 = feat_pool.tile([P, N], F32)
        F3 = feat_pool.tile([P, N], F32)
        F4a = feat_pool.tile([P, N], F32)
        F4b = feat_pool.tile([P, N], F32)
        nxr_t = [feat_pool.tile([P, N], F32, tag=f"nxr{i % 2}", name=f"nxr_t{i}")
                 for i in range(4)]
        xs_pool = ctx.enter_context(tc.tile_pool(name="xs", bufs=1))

        l1_pool = ctx.enter_context(tc.tile_pool(name="l1in", bufs=1))
        X1 = l1_pool.tile([P, N], F32)
        nc.sync.dma_start(out=X1[0:3, :], in_=x_d[:, :])
        XS1 = xs_pool.tile([P, N], F32, tag="XS")
        nc.scalar.mul(XS1[0:3, :], X1[0:3, :], 2.0)
        xxc = l1_pool.tile([P, NB], F32)
        xT_d = x_d.rearrange("c n -> n c")
        with tc.tile_pool(name="l1xt", bufs=2) as xtp:
            for b in range(NB):
                xt = xtp.tile([P, 3], F32, tag="xt")
                nc.sync.dma_start(out=xt, in_=xT_d[b * P:(b + 1) * P, :])
                sq = xtp.tile([P, 3], F32, tag="sq")
                nc.scalar.activation(out=sq, in_=xt, func=AF.Square,
                                     accum_out=xxc[:, b:b + 1])
        nc.vector.tensor_scalar(out=xxc, in0=xxc, scalar1=-1.0, scalar2=None,
                                op0=ALU.mult)
        with tc.tile_pool(name="l1tr", bufs=1) as trp, \
             tc.tile_pool(name="l1trp", bufs=1, space="PSUM") as trpp:
            tp = trpp.tile([NB, P], F32, space="PSUM")
            nc.tensor.transpose(out=tp, in_=xxc, identity=ident)
            tps = trp.tile([NB, P], F32)
            nc.scalar.copy(out=tps, in_=tp)
            nc.sync.dma_start(out=nxr_t[0][0:1, :], in_=tps[:, :])

        Xin = X1[0:3, :]
        XSin = XS1[0:3, :]
        nxr = nxr_t[0]

        out_tiles = [
            [(F12, 0)],
            [(F12, 64)],
            [(F3, 0)],
            [(F4a, 0), (F4b, 0)],
        ]

        for li, (C, O, _, _, _) in enumerate(LAY):
            with ExitStack() as lay_ctx:
                lp = lay_ctx.enter_context(
                    tc.tile_pool(name=f"l{li}sb", bufs=1))
                dr = lay_ctx.enter_context(
                    tc.tile_pool(name=f"l{li}dr", bufs=1, space="DRAM"))

                WaT = lp.tile([P, O], F32)
                nc.sync.dma_start(out=WaT[0:C, :], in_=c_WaT[li][:, :])
                trow = lp.tile([P, O], F32)
                nc.sync.dma_start(out=trow[0:1, :], in_=c_trow[li][:, :])
                WdT = lp.tile([P, O], F32)
                nc.sync.dma_start(out=WdT[0:C, :], in_=c_WdT[li][:, :])

                bmT_d = dr.tile([N, O], F32, name=f"bmT_d{li}")
                aT_sb = [lp.tile([P, O], F32, tag=f"aT{b}",
                                 name=f"aT_sb{li}_{b}") for b in range(NB)]

                # phase 1: bmT -> DRAM, aT -> SBUF
                with tc.tile_pool(name=f"l{li}p1", bufs=2, space="PSUM") as p1, \
                     tc.tile_pool(name=f"l{li}p1s", bufs=3) as p1s:
                    for b in range(NB):
                        sl = slice(b * P, (b + 1) * P)
                        pb = p1.tile([P, O], F32, space="PSUM", tag="pb")
                        nc.tensor.matmul(out=pb, lhsT=Xin[:, sl],
                                         rhs=WdT[0:C, :], start=True, stop=True)
                        sbt = p1s.tile([P, O], F32, tag="sbt")
                        nc.scalar.copy(out=sbt, in_=pb)
                        nc.sync.dma_start(out=bmT_d[sl, :], in_=sbt)
                        pa = p1.tile([P, O], F32, space="PSUM", tag="pa")
                        nc.tensor.matmul(out=pa, lhsT=Xin[:, sl],
                                         rhs=WaT[0:C, :], start=True, stop=False)
                        nc.tensor.matmul(out=pa, lhsT=ones_t[0:1, sl],
                                         rhs=trow[0:1, :], start=False, stop=True)
                        nc.scalar.copy(out=aT_sb[b], in_=pa)

                # phase 2a: sel matmul + top-20 for all blocks
                Y_sb = [lp.tile([P, O], F32, tag=f"Y{b}",
                                name=f"Y_sb{li}_{b}") for b in range(NB)]
                idx_t = [lp.tile([P, 24], U32, tag=f"idx{b}",
                                 name=f"idx{li}_{b}") for b in range(NB)]
                xxn = lp.tile([P, NB], F32)
                with tc.tile_pool(name=f"l{li}p2", bufs=2, space="PSUM") as p2, \
                     tc.tile_pool(name=f"l{li}p2s", bufs=3) as p2s:
                    for b in range(NB):
                        sl = slice(b * P, (b + 1) * P)
                        gp = p2.tile([P, N], F32, space="PSUM", tag="gp")
                        for c in range(N // CH):
                            cs = slice(c * CH, (c + 1) * CH)
                            nc.tensor.matmul(out=gp[:, cs], lhsT=Xin[:, sl],
                                             rhs=XSin[:, cs],
                                             start=True, stop=False)
                            nc.tensor.matmul(out=gp[:, cs],
                                             lhsT=ones_t[0:1, sl],
                                             rhs=nxr[0:1, cs],
                                             start=False, stop=True)
                        mx = p2s.tile([P, 8], F32, tag="mx")
                        idx = idx_t[b]
                        nc.vector.max(out=mx, in_=gp)
                        nc.vector.max_index(out=idx[:, 0:8], in_max=mx,
                                            in_values=gp)
                        nc.vector.match_replace(out=gp, in_to_replace=mx,
                                                in_values=gp, imm_value=-1e30)
                        nc.vector.max(out=mx, in_=gp)
                        nc.vector.max_index(out=idx[:, 8:16], in_max=mx,
                                            in_values=gp)
                        nc.vector.match_replace(out=gp, in_to_replace=mx,
                                                in_values=gp, imm_value=-1e30)
                        nc.vector.max(out=mx, in_=gp)
                        nc.vector.max_index(out=idx[:, 16:24], in_max=mx,
                                            in_values=gp)

                    # fence: issue gathers only after all top-k DVE work
                    bar = p2s.tile([P, 24], U32, tag="bar")
                    nc.gpsimd.tensor_copy(out=bar, in_=idx_t[NB - 1])

                    # phase 2b: gathers + z + lrelu
                    for b in range(NB):
                        idx = idx_t[b]
                        z = p2s.tile([P, O], F32, tag="z")
                        for r in range(K):
                            gt = p2s.tile([P, O], F32, tag="gt", bufs=6,
                                          name=f"gt{b}_{r}")
                            nc.gpsimd.indirect_dma_start(
                                out=gt[:, :], out_offset=None,
                                in_=bmT_d[:, :],
                                in_offset=bass.IndirectOffsetOnAxis(
                                    ap=idx[:, r:r + 1], axis=0))
                            if r == 0:
                                nc.vector.tensor_copy(out=z, in_=gt)
                            else:
                                nc.vector.tensor_tensor(out=z, in0=z, in1=gt,
                                                        op=ALU.max)
                        nc.vector.tensor_tensor(out=z, in0=z, in1=aT_sb[b],
                                                op=ALU.add)
                        zs = p2s.tile([P, O], F32, tag="zs")
                        nc.vector.tensor_scalar(out=zs, in0=z, scalar1=0.2,
                                                scalar2=None, op0=ALU.mult)
                        nc.vector.tensor_tensor(out=Y_sb[b], in0=z, in1=zs,
                                                op=ALU.max)
                        sq = p2s.tile([P, O], F32, tag="sq")
                        nc.scalar.activation(out=sq, in_=Y_sb[b],
                                             func=AF.Square,
                                             accum_out=xxn[:, b:b + 1])

                # phase 3: transposes + next -xx row
                nxt = out_tiles[li]
                with tc.tile_pool(name=f"l{li}p3", bufs=2, space="PSUM") as p3:
                    for b in range(NB):
                        for h in range(O // P if O >= P else 1):
                            if O <= P:
                                src = Y_sb[b][:, :]
                                dst_t, dst_r = nxt[0]
                                rows = O
                            else:
                                src = Y_sb[b][:, h * P:(h + 1) * P]
                                dst_t, dst_r = nxt[h]
                                rows = P
                            tp = p3.tile([P, P], F32, space="PSUM", tag="tp")
                            nc.tensor.transpose(out=tp[0:rows, :], in_=src,
                                                identity=ident)
                            nc.scalar.copy(
                                out=dst_t[dst_r:dst_r + rows,
                                          b * P:(b + 1) * P],
                                in_=tp[0:rows, :])
                    if li < 3:
                        nc.vector.tensor_scalar(out=xxn, in0=xxn, scalar1=-1.0,
                                                scalar2=None, op0=ALU.mult)
                        tpx = p3.tile([NB, P], F32, space="PSUM", tag="tpx")
                        nc.tensor.transpose(out=tpx, in_=xxn, identity=ident)
                        tps = lp.tile([NB, P], F32)
                        nc.scalar.copy(out=tps, in_=tpx)
                        nc.sync.dma_start(out=nxr_t[li + 1][0:1, :],
                                          in_=tps[:, :])

            if li == 0:
                Xin = F12[0:64, :]
            elif li == 1:
                F2c = xs_pool.tile([P, N], F32, tag="F2c", name="F2c")
                nc.scalar.copy(out=F2c[0:64, :], in_=F12[64:128, :])
                Xin = F2c[0:64, :]
            elif li == 2:
                Xin = F3[:, :]
            else:
                break
            nxr = nxr_t[li + 1]
            Cn = Xin.shape[0]
            XSt = xs_pool.tile([P, N], F32, tag="XS")
            nc.scalar.mul(XSt[0:Cn, :], Xin, 2.0)
            XSin = XSt[0:Cn, :]

        # W5 + pooling
        fc_pool = ctx.enter_context(tc.tile_pool(name="fc", bufs=1))
        fmax = fc_pool.tile([P, 8], F32)
        fsum = fc_pool.tile([P, 8], F32)
        with tc.tile_pool(name="w5sb", bufs=1) as w5s, \
             tc.tile_pool(name="w5ps", bufs=2, space="PSUM") as w5p, \
             tc.tile_pool(name="w5tmp", bufs=3) as w5t:
            W5sb = w5s.tile([P, 4, 1024], F32)
            nc.sync.dma_start(out=W5sb,
                              in_=c_W5T.rearrange("(a p) o -> p a o", p=P))
            t5sb = w5s.tile([P, 16], F32)
            nc.sync.dma_start(out=t5sb, in_=c_t5[:, :])
            KCH = [F12, F3, F4a, F4b]
            for ob in range(8):
                emb = w5t.tile([P, N], F32, tag="emb")
                for c in range(N // CH):
                    cs = slice(c * CH, (c + 1) * CH)
                    pw = w5p.tile([P, CH], F32, space="PSUM", tag="pw")
                    for kc in range(4):
                        nc.tensor.matmul(
                            out=pw, lhsT=W5sb[:, kc, ob * P:(ob + 1) * P],
                            rhs=KCH[kc][:, cs], start=(kc == 0), stop=(kc == 3))
                    z02 = w5t.tile([P, CH], F32, tag="z02")
                    nc.vector.tensor_scalar(out=z02, in0=pw,
                                            scalar1=t5sb[:, ob:ob + 1],
                                            scalar2=0.2, op0=ALU.add,
                                            op1=ALU.mult)
                    r08 = w5t.tile([P, CH], F32, tag="r08")
                    nc.scalar.activation(out=r08, in_=pw, func=AF.Relu,
                                         bias=t5sb[:, 8 + ob:9 + ob], scale=0.8)
                    nc.vector.tensor_tensor(out=emb[:, cs], in0=z02, in1=r08,
                                            op=ALU.add)
                nc.vector.tensor_reduce(out=fmax[:, ob:ob + 1], in_=emb,
                                        axis=mybir.AxisListType.X, op=ALU.max)
                nc.vector.tensor_reduce(out=fsum[:, ob:ob + 1], in_=emb,
                                        axis=mybir.AxisListType.X, op=ALU.add)

        # FC chain
        with tc.tile_pool(name="fcsb", bufs=1) as fs, \
             tc.tile_pool(name="fcps", bufs=2, space="PSUM") as fp:
            L1sb = fs.tile([P, 16, 512], F32)
            nc.sync.dma_start(out=L1sb,
                              in_=c_L1T.rearrange("(a p) o -> p a o", p=P))
            t6sb = fs.tile([P, 512], F32)
            nc.sync.dma_start(out=t6sb[0:1, :], in_=c_t6[:, :])
            h1 = fs.tile([P, 4], F32)
            for ob in range(4):
                ph = fp.tile([P, 1], F32, space="PSUM", tag="ph")
                for kc in range(16):
                    rhs = fmax[:, kc:kc + 1] if kc < 8 else fsum[:, kc - 8:kc - 7]
                    nc.tensor.matmul(out=ph,
                                     lhsT=L1sb[:, kc, ob * P:(ob + 1) * P],
                                     rhs=rhs, start=(kc == 0), stop=False)
                nc.tensor.matmul(out=ph, lhsT=t6sb[0:1, ob * P:(ob + 1) * P],
                                 rhs=ones11, start=False, stop=True)
                zs = fs.tile([P, 1], F32, tag="h1zs")
                nc.vector.tensor_scalar(out=zs, in0=ph, scalar1=0.2,
                                        scalar2=None, op0=ALU.mult)
                nc.vector.tensor_tensor(out=h1[:, ob:ob + 1], in0=ph, in1=zs,
                                        op=ALU.max)
            L2sb = fs.tile([P, 4, 256], F32)
            nc.sync.dma_start(out=L2sb,
                              in_=c_L2T.rearrange("(a p) o -> p a o", p=P))
            t7sb = fs.tile([P, 256], F32)
            nc.sync.dma_start(out=t7sb[0:1, :], in_=c_t7[:, :])
            h2 = fs.tile([P, 2], F32)
            for ob in range(2):
                ph = fp.tile([P, 1], F32, space="PSUM", tag="ph2")
                for kc in range(4):
                    nc.tensor.matmul(out=ph,
                                     lhsT=L2sb[:, kc, ob * P:(ob + 1) * P],
                                     rhs=h1[:, kc:kc + 1],
                                     start=(kc == 0), stop=False)
                nc.tensor.matmul(out=ph, lhsT=t7sb[0:1, ob * P:(ob + 1) * P],
                                 rhs=ones11, start=False, stop=True)
                zs = fs.tile([P, 1], F32, tag="h2zs")
                nc.vector.tensor_scalar(out=zs, in0=ph, scalar1=0.2,
                                        scalar2=None, op0=ALU.mult)
                nc.vector.tensor_tensor(out=h2[:, ob:ob + 1], in0=ph, in1=zs,
                                        op=ALU.max)
            L3sb = fs.tile([P, 2, 40], F32)
            nc.sync.dma_start(out=L3sb,
                              in_=c_L3T.rearrange("(a p) o -> p a o", p=P))
            L3bsb = fs.tile([P, 40], F32)
            nc.sync.dma_start(out=L3bsb[0:1, :], in_=c_L3b[:, :])
            po = fp.tile([40, 1], F32, space="PSUM", tag="po")
            for kc in range(2):
                nc.tensor.matmul(out=po, lhsT=L3sb[:, kc, :],
                                 rhs=h2[:, kc:kc + 1],
                                 start=(kc == 0), stop=False)
            nc.tensor.matmul(out=po, lhsT=L3bsb[0:1, :], rhs=ones11,
                             start=False, stop=True)
            osb = fs.tile([40, 1], F32)
            nc.scalar.copy(out=osb, in_=po)
            nc.sync.dma_start(out=out_d[:, :], in_=osb)


# --------------------------------------------------------------------------
# Cached PJRT runner (compiles once, reuses the jitted shard_map)
# --------------------------------------------------------------------------

def _make_runner(nc, n_cores):
    import jax
    from jax.sharding import Mesh, PartitionSpec
    import concourse.mybir as mybir
    from concourse import bass2jax as B2J

    B2J.install_neuronx_cc_hook()
    dbg_name = None
    if nc.dbg_addr is not None:
        assert not nc.dbg_callbacks
        dbg_name = nc.dbg_addr.name
    partition_name = (nc.partition_id_tensor.name
                      if nc.partition_id_tensor else None)

    in_names, out_names, out_avals, zero_outs = [], [], [], []
    for alloc in nc.m.functions[0].allocations:
        if not isinstance(alloc, mybir.MemoryLocationSet):
            continue
        name = alloc.memorylocations[0].name
        if alloc.kind == "ExternalInput":
            if name != partition_name:
                in_names.append(name)
        elif alloc.kind == "ExternalOutput":
            shape = tuple(alloc.tensor_shape)
            dtype = mybir.dt.np(alloc.dtype)
            out_names.append(name)
            out_avals.append(jax.core.ShapedArray(shape, dtype))
            zero_outs.append(np.zeros(shape, dtype))
    n_params = len(in_names)
    n_outs = len(out_avals)
    all_in_names = list(in_names) + list(out_names)
    if partition_name is not None:
        all_in_names.append(partition_name)
    donate = tuple(range(n_params, n_params + n_outs))

    def _body(*args):
        operands = list(args)
        if partition_name is not None:
            operands.append(B2J.partition_id_tensor())
        outs = B2J._bass_exec_p.bind(
            *operands,
            out_avals=tuple(out_avals),
            in_names=tuple(all_in_names),
            out_names=tuple(out_names),
            lowering_input_output_aliases=(),
            sim_require_finite=True,
            sim_require_nnan=True,
            nc=nc,
        )
        return tuple(outs)

    devices = jax.devices()[:n_cores]
    mesh = Mesh(np.asarray(devices), ("core",))
    in_specs = (PartitionSpec("core"),) * (n_params + n_outs)
    out_specs = (PartitionSpec("core"),) * n_outs
    sharded = jax.jit(
        jax.shard_map(_body, mesh=mesh, in_specs=in_specs,
                      out_specs=out_specs, check_vma=False),
        donate_argnums=donate, keep_unused=True,
    )

    def run(in_maps):
        maps = in_maps
        if dbg_name is not None:
            zdbg = np.zeros((1, 2), np.uint32)
            maps = [{**m, dbg_name: zdbg} for m in maps]
        concat_in = [
            np.concatenate([np.asarray(maps[c][nm]) for c in range(n_cores)],
                           axis=0)
            for nm in in_names
        ]
        concat_zeros = [
            np.zeros((n_cores * z.shape[0], *z.shape[1:]), z.dtype)
            for z in zero_outs
        ]
        out_arrs = sharded(*concat_in, *concat_zeros)
        return [
            {nm: np.asarray(out_arrs[i]).reshape(n_cores,
                                                 *out_avals[i].shape)[c]
             for i, nm in enumerate(out_names)}
            for c in range(n_cores)
        ]

    return run


def _build_bass(w):
    import concourse.mybir as mybir
    from concourse import bacc, tile

    nc = bacc.Bacc("TRN2", target_bir_lowering=False, debug=False)
    x_d = nc.dram_tensor("x", (3, N), mybir.dt.float32, kind="ExternalInput")
    out_d = nc.dram_tensor("out", (40, 1), mybir.dt.float32,
                           kind="ExternalOutput")
    with tile.TileContext(nc) as tc:
        _build_tc(tc, w, x_d[:, :], out_d[:, :])
    nc.compile()
    return _make_runner(nc, N_CORES)


# --------------------------------------------------------------------------
# numpy reference (self-check of the Bass path on first call)
# --------------------------------------------------------------------------

def _numpy_forward_one(x, w):
    def lrelu(v):
        return np.where(v > 0, v, 0.2 * v)

    def edgeconv(xc, Wm, i):
        C = Wm.shape[1] // 2
        s, t = _fold_bn(w, i)
        xt = xc.T.astype(np.float32)
        xx = np.sum(xc * xc, axis=0, dtype=np.float32)
        d = xx[:, None] + xx[None, :] - 2.0 * (xt @ xt.T)
        idx = np.argpartition(d, K - 1, axis=1)[:, :K]
        A = (Wm[:, :C] - Wm[:, C:]) * s[:, None]
        D = Wm[:, C:] * s[:, None]
        a = (A @ xc).astype(np.float32)
        bm = (D @ xc).astype(np.float32)
        B = bm.T[idx].max(axis=1).T
        return lrelu(a + B + t[:, None]).astype(np.float32)

    x1 = edgeconv(x, np.asarray(w["W1"], np.float64), 1)
    x2 = edgeconv(x1, np.asarray(w["W2"], np.float64), 2)
    x3 = edgeconv(x2, np.asarray(w["W3"], np.float64), 3)
    x4 = edgeconv(x3, np.asarray(w["W4"], np.float64), 4)
    xc = np.concatenate([x1, x2, x3, x4], axis=0)
    s5, t5 = _fold_bn(w, 5)
    emb = lrelu((np.asarray(w["W5"], np.float64) * s5[:, None]) @ xc
                + t5[:, None])
    feat = np.concatenate([emb.max(axis=1), emb.mean(axis=1)])
    s6, t6 = _fold_bn(w, 6)
    h = lrelu((np.asarray(w["L1"], np.float64) * s6[:, None]) @ feat + t6)
    s7, t7 = _fold_bn(w, 7)
    h = lrelu((np.asarray(w["L2"], np.float64) * s7[:, None]) @ h + t7)
    return (np.asarray(w["L3"], np.float64) @ h
            + np.asarray(w["L3_b"], np.float64)).astype(np.float32)


# --------------------------------------------------------------------------
# jax fallback (constant-folded weights, pmap)
# --------------------------------------------------------------------------

def _build_jax(w):
    import jax
    import jax.numpy as jnp

    C_ = {}
    all_pos = True
    for i, wk in [(1, "W1"), (2, "W2"), (3, "W3"), (4, "W4")]:
        Wm = np.asarray(w[wk], np.float32)
        C = Wm.shape[1] // 2
        s, t = _fold_bn(w, i)
        s = s.astype(np.float32)
        t = t.astype(np.float32)
        all_pos &= bool(np.min(s) > 0)
        C_[f"A{i}"] = jnp.asarray(Wm[:, :C] - Wm[:, C:])
        C_[f"D{i}"] = jnp.asarray(Wm[:, C:])
        C_[f"s{i}"] = jnp.asarray(s)
        C_[f"t{i}"] = jnp.asarray(t)
    for i, wk in [(5, "W5"), (6, "L1"), (7, "L2")]:
        s, t = _fold_bn(w, i)
        C_[f"M{i}"] = jnp.asarray(
            (np.asarray(w[wk], np.float64) * s[:, None]).astype(np.float32))
        C_[f"t{i}"] = jnp.asarray(t.astype(np.float32))
    C_["L3"] = jnp.asarray(np.asarray(w["L3"], np.float32))
    C_["L3_b"] = jnp.asarray(np.asarray(w["L3_b"], np.float32))

    def lrelu(y):
        return jnp.where(y > 0, y, 0.2 * y)

    def edgeconv(x, i):
        xt = x.T
        xx = jnp.sum(x * x, axis=0)
        d = xx[:, None] + xx[None, :] - 2.0 * (xt @ xt.T)
        _, idx = jax.lax.top_k(-d, K)
        a = C_[f"A{i}"] @ x
        bm = C_[f"D{i}"] @ x
        nbr = bm.T[idx]
        s = C_[f"s{i}"]
        if all_pos:
            B = jnp.max(nbr, axis=1).T
        else:
            B = jnp.where((s >= 0)[:, None], jnp.max(nbr, axis=1).T,
                          jnp.min(nbr, axis=1).T)
        return lrelu((a + B) * s[:, None] + C_[f"t{i}"][:, None])

    def fwd(x):
        x1 = edgeconv(x, 1)
        x2 = edgeconv(x1, 2)
        x3 = edgeconv(x2, 3)
        x4 = edgeconv(x3, 4)
        xc = jnp.concatenate([x1, x2, x3, x4], axis=0)
        emb = lrelu(C_["M5"] @ xc + C_["t5"][:, None])
        feat = jnp.concatenate([jnp.max(emb, axis=1), jnp.mean(emb, axis=1)])
        h = lrelu(C_["M6"] @ feat + C_["t6"])
        h = lrelu(C_["M7"] @ h + C_["t7"])
        return C_["L3"] @ h + C_["L3_b"]

    pm = jax.pmap(fwd, devices=jax.devices()[:N_CORES])

    def run_jax(x):
        return np.asarray(pm(x)).astype(np.float32)

    return run_jax


# --------------------------------------------------------------------------
# entry point
# --------------------------------------------------------------------------

_CACHE = {}


def _fingerprint(arrs):
    h = 0
    for a in arrs:
        h ^= hash((a.shape, a.dtype.str, a.tobytes()[:64], a.tobytes()[-64:]))
    return h


def _get_impl(w, x):
    try:
        run_bass = _build_bass(w)
        in_maps = [{"x": np.ascontiguousarray(x[i])} for i in range(N_CORES)]
        outs = run_bass(in_maps)
        got = outs[0]["out"].reshape(40)
        ref = _numpy_forward_one(x[0].astype(np.float32), w)
        err = np.abs(got - ref).max() / max(np.abs(ref).max(), 1e-30)
        if not np.isfinite(err) or err > 5e-3:
            raise RuntimeError(f"bass self-check failed: rel err {err:.3e}")

        def run(x_full):
            maps = [{"x": np.ascontiguousarray(x_full[i])}
                    for i in range(N_CORES)]
            res = run_bass(maps)
            return np.stack([r["out"].reshape(40) for r in res]).astype(
                np.float32)

        return run
    except Exception:
        run_jax = _build_jax(w)
        return run_jax


def kernel(**inputs):
    x = np.ascontiguousarray(np.asarray(inputs["x"], dtype=np.float32))
    assert x.shape[0] == N_CORES, f"expected batch {N_CORES}, got {x.shape}"
    host_w = [np.ascontiguousarray(np.asarray(inputs[k], dtype=np.float32))
              for k in _WEIGHT_KEYS]
    fp = _fingerprint(host_w)
    if fp not in _CACHE:
        w = {k: v for k, v in zip(_WEIGHT_KEYS, host_w)}
        _CACHE[fp] = _get_impl(w, x)
    return _CACHE[fp](x)
